# revision 31
# baseline (speedup 1.0000x reference)
"""2-layer GraphSAGE (mean aggregation) over 8 TRN2 NeuronCores.

Design (descriptor-generation-bound problem; ~8ns/idx SWDGE emission is the
hard floor, so everything else hides under it):
  - Destination-shard nodes across 8 cores (contiguous 6250-node ranges). x is
    replicated into every core's DRAM at input-load time, so layer 1 needs no
    communication.
  - Host (numpy, uncounted) packs the gather tables two-nodes-per-row
    ([25001, 256] bf16, 512B rows), so idx = node//2 fits dma_gather's signed
    int16 indices with a single table and no source-half split. Edges are
    sorted per core by destination window (128 nodes); each window's slot
    list pads to a multiple of 128 (~5%); slots whose two parities serve two
    edges of the same window merge.
  - Device, per 128-edge chunk: dma_gather (non-transpose, batched 4096-idx
    calls) lands paired source rows as [128 edge-lanes, slab, 256]; DVE
    builds one-hot [edge, dst-lane] matrices per parity via is_equal against
    a constant iota (dst-lane 255 masks pad slots); PE matmul-accumulates
    agg[dst, feat] in PSUM over the window's chunks. Mean = agg * 1/deg
    (DVE). PE transposes mean (identity matmul), then rows = mean @ W_l +
    x @ W_r + 1*b (bias as a K=1 rank-1 matmul). ACT applies relu, DVE the
    dropout mask. h rows go to an internal DRAM buffer; a PE transpose keeps
    h^T resident in SBUF for layer 2's W_r term.
  - One bf16 AllGather exchanges h rows between layers (the only collective);
    layer 2 gathers from the gathered table (viewed as 512B pair-rows, with
    all-zero pad ranks as sentinels) and writes output rows; the host
    reassembles core slices.
"""

import sys

for _p in ("/opt/trn_rl_repo",):
    if _p not in sys.path:
        sys.path.insert(0, _p)

import hashlib
import numpy as np
import ml_dtypes

BF16 = ml_dtypes.bfloat16
P = 128
D = 128

SLAB_BUDGET = 32  # max 128-edge chunks per gather call (4096 idxs, 1 MB bf16)


# --------------------------------------------------------------------------
# Host-side schedule construction
# --------------------------------------------------------------------------

def build_meta(src, dst, N, ncores):
    src = np.asarray(src, np.int64)
    dst = np.asarray(dst, np.int64)
    npc = N // ncores
    assert npc * ncores == N, (N, ncores)
    blocks = -(-npc // P)
    npad = blocks * P
    assert npc < npad, "need pad ranks for layer-2 zero sentinels"
    assert ncores % 2 == 0
    split1 = N // 2
    split2 = (ncores // 2) * npad
    assert split1 + 1 <= 32768 and (N - split1) + 1 <= 32768
    assert split2 <= 32768

    deg = np.bincount(dst, minlength=N)
    invcnt = (1.0 / np.maximum(deg, 1.0)).astype(np.float32)

    perm = -np.ones((ncores, npad), np.int64)
    for c in range(ncores):
        perm[c, :npc] = np.arange(c * npc, (c + 1) * npc)
    rank = np.mod(np.arange(N), npc)
    core_of = np.arange(N) // npc
    pos = core_of * npad + rank  # row in the allgathered h table
    npc_sent = npc

    layers = []
    idx_arrays = [[None] * ncores, [None] * ncores]
    dstl_arrays = [[None] * ncores, [None] * ncores]
    for L in range(2):
        # paired tables: row j holds nodes (2j, 2j+1); idx = key//2 fits int16
        sent = (N // 2) if L == 0 else (npc // 2)
        nrows = (N // 2) + 1 if L == 0 else (ncores * npad) // 2

        nch = np.zeros(blocks, np.int64)
        per_core = []
        for c in range(ncores):
            sel = core_of[dst] == c
            s_c = src[sel]
            r = rank[dst[sel]]
            key = s_c if L == 0 else pos[s_c]
            pairv = key >> 1
            parv = key & 1
            w = r // P
            dl = r % P
            # within-(w,pair,par) occurrence counter k
            okey = (((w << 15) | pairv) << 1) | parv
            order = np.argsort(okey, kind="stable")
            sk = okey[order]
            change = np.empty(len(sk), bool)
            if len(sk):
                change[0] = True
                change[1:] = sk[1:] != sk[:-1]
            first_occ = np.nonzero(change)[0]
            gid = np.cumsum(change) - 1
            k = np.arange(len(sk)) - first_occ[gid]
            assert len(k) == 0 or k.max() < 256
            # slot identity: (w, pair, k); shared between parities
            wpair = sk >> 1
            slotkey = (wpair << 8) | k
            uslot = np.unique(slotkey)
            uw = (uslot >> 23).astype(np.int64)
            cntw = np.bincount(uw, minlength=blocks)
            np.maximum(nch, -(-cntw // P), out=nch)
            per_core.append((order, slotkey, uslot, uw, w, dl, parv, pairv))
        nch = np.maximum(nch, 1)

        # group windows under the slab budget
        groups_w = []
        cur, ca = [], 0
        for b in range(blocks):
            a = int(nch[b])
            if cur and ca + a > SLAB_BUDGET:
                groups_w.append(cur)
                cur, ca = [], 0
            cur.append(b)
            ca += a
        groups_w.append(cur)

        total_slabs = int(nch.sum())
        slab_of = np.zeros(blocks, np.int64)
        groups = []
        off = 0
        for gw in groups_w:
            ginfo = {"windows": []}
            g0 = off
            for b in gw:
                slab_of[b] = off
                off += int(nch[b])
            ginfo["col"] = g0 * 8
            ginfo["NI"] = (off - g0) * P
            ginfo["base"] = g0
            for b in gw:
                chunks = []
                for j in range(int(nch[b])):
                    gslab = int(slab_of[b]) + j
                    chunks.append((gslab - ginfo["base"], gslab))
                ginfo["windows"].append({"w": b, "chunks": chunks})
            groups.append(ginfo)
        assert off == total_slabs

        for c in range(ncores):
            order, slotkey, uslot, uw, w, dl, parv, pairv = per_core[c]
            flat = np.full(total_slabs * P, sent, np.int16)
            dst2 = np.full((total_slabs * 2, P), 255, np.int16)
            # slot positions: uslot sorted; per window contiguous runs
            wfirst = np.searchsorted(uw, np.arange(blocks))
            spos = slab_of[uw] * P + (np.arange(len(uslot)) - wfirst[uw])
            upair = (uslot >> 8) & ((1 << 15) - 1)
            flat[spos] = upair.astype(np.int16)
            # per-edge: locate slot, write dstl into (chunk, parity, lane)
            es = np.searchsorted(uslot, slotkey)  # edges in `order` order
            eposn = spos[es]
            echunk = eposn // P
            elane = eposn % P
            epar = parv[order]
            edl = dl[order]
            dst2[2 * echunk + epar, elane] = edl.astype(np.int16)
            idx_arrays[L][c] = np.ascontiguousarray(
                np.tile(flat.reshape(-1, 16).T, (8, 1)))
            dstl_arrays[L][c] = np.ascontiguousarray(dst2.T.astype(BF16))

        layers.append({"groups": groups, "C": total_slabs * 8,
                       "TC": total_slabs, "nrows": nrows})

    return {
        "N": N, "ncores": ncores, "blocks": blocks, "npad": npad,
        "split1": split1, "split2": split2,
        "perm": perm, "invcnt": invcnt,
        "layers": layers, "idx": idx_arrays, "dstl": dstl_arrays,
        "tabA1": split1 + 1, "tabB1": (N - split1) + 1,
    }


# --------------------------------------------------------------------------
# Bass graph
# --------------------------------------------------------------------------

def build_nc(meta):
    from concourse import bacc, mybir
    from concourse.tile import TileContext
    from concourse.masks import make_identity

    dt = mybir.dt
    ALU = mybir.AluOpType
    AF = mybir.ActivationFunctionType
    m = meta
    npad, ncores, blocks = m["npad"], m["ncores"], m["blocks"]

    nc = bacc.Bacc()

    x2 = nc.declare_dram_parameter("x2", [m["layers"][0]["nrows"], 2 * D], dt.bfloat16, isOutput=False)
    idx1 = nc.declare_dram_parameter("idx1", [P, m["layers"][0]["C"]], dt.int16, isOutput=False)
    idx2 = nc.declare_dram_parameter("idx2", [P, m["layers"][1]["C"]], dt.int16, isOutput=False)
    dstl1 = nc.declare_dram_parameter("dstl1", [P, 2 * m["layers"][0]["TC"]], dt.bfloat16, isOutput=False)
    dstl2 = nc.declare_dram_parameter("dstl2", [P, 2 * m["layers"][1]["TC"]], dt.bfloat16, isOutput=False)
    xT = nc.declare_dram_parameter("xT", [P, npad], dt.bfloat16, isOutput=False)
    m2r = nc.declare_dram_parameter("m2r", [npad, D], dt.bfloat16, isOutput=False)
    invc = nc.declare_dram_parameter("invc", [P, blocks], dt.float32, isOutput=False)
    iota = nc.declare_dram_parameter("iota", [P, P], dt.bfloat16, isOutput=False)
    onesr = nc.declare_dram_parameter("onesr", [1, P], dt.bfloat16, isOutput=False)
    w1l = nc.declare_dram_parameter("w1l", [P, P], dt.bfloat16, isOutput=False)
    w1r = nc.declare_dram_parameter("w1r", [P, P], dt.bfloat16, isOutput=False)
    w2l = nc.declare_dram_parameter("w2l", [P, P], dt.bfloat16, isOutput=False)
    w2r = nc.declare_dram_parameter("w2r", [P, P], dt.bfloat16, isOutput=False)
    b1r = nc.declare_dram_parameter("b1r", [1, P], dt.bfloat16, isOutput=False)
    b2r = nc.declare_dram_parameter("b2r", [1, P], dt.bfloat16, isOutput=False)
    out = nc.declare_dram_parameter("out", [npad, D], dt.float32, isOutput=True)

    cc_in = nc.dram_tensor("cc_in", [npad, D], dt.bfloat16)
    h_full = nc.dram_tensor("h_full", [ncores * npad, D], dt.bfloat16, addr_space="Shared")

    with TileContext(nc) as tc:
        with (
            tc.tile_pool(name="persist", bufs=1) as pers,
            tc.tile_pool(name="work", bufs=4) as wp,
            tc.tile_pool(name="oh", bufs=8) as ohp,
            tc.tile_pool(name="gath", bufs=3) as gp,
            tc.tile_pool(name="psagg", bufs=3, space="PSUM") as psa,
            tc.tile_pool(name="pstr", bufs=2, space="PSUM") as pst,
            tc.tile_pool(name="psout", bufs=2, space="PSUM") as pso,
        ):
            def load(dram, shape, dtype, tag):
                t = pers.tile(shape, dtype, tag=tag)
                nc.sync.dma_start(out=t[:], in_=dram[:])
                return t

            def load_split(dram, C, tag, c0, c1):
                t = pers.tile([P, C], dt.int16, tag=tag)
                nc.sync.dma_start(out=t[:, c0:c1], in_=dram[:, c0:c1])
                if c0 > 0:
                    nc.sync.dma_start(out=t[:, 0:c0], in_=dram[:, 0:c0])
                if c1 < C:
                    nc.sync.dma_start(out=t[:, c1:C], in_=dram[:, c1:C])
                return t

            g0s = [max(m["layers"][L]["groups"], key=lambda gg: len(gg["windows"]))
                   for L in range(2)]
            idx_sb = [
                load_split(idx1, m["layers"][0]["C"], "idx1",
                           g0s[0]["col"], g0s[0]["col"] + g0s[0]["NI"] // 16),
                load_split(idx2, m["layers"][1]["C"], "idx2",
                           g0s[1]["col"], g0s[1]["col"] + g0s[1]["NI"] // 16),
            ]
            dstl_sb = [
                load(dstl1, [P, 2 * m["layers"][0]["TC"]], dt.bfloat16, "dstl1"),
                load(dstl2, [P, 2 * m["layers"][1]["TC"]], dt.bfloat16, "dstl2"),
            ]
            xT_sb = load(xT, [P, npad], dt.bfloat16, "xT")
            invc_sb = load(invc, [P, blocks], dt.float32, "invc")
            iota_sb = load(iota, [P, P], dt.bfloat16, "iota")
            ones_sb = load(onesr, [1, P], dt.bfloat16, "ones")
            w_sb = [
                (load(w1l, [P, P], dt.bfloat16, "w1l"), load(w1r, [P, P], dt.bfloat16, "w1r"),
                 load(b1r, [1, P], dt.bfloat16, "b1r")),
                (load(w2l, [P, P], dt.bfloat16, "w2l"), load(w2r, [P, P], dt.bfloat16, "w2r"),
                 load(b2r, [1, P], dt.bfloat16, "b2r")),
            ]
            ident = pers.tile([P, P], dt.bfloat16, tag="ident")
            make_identity(nc, ident[:])
            hT_sb = pers.tile([P, npad], dt.bfloat16, tag="hT")

            for L in range(2):
                lm = m["layers"][L]
                if L == 0:
                    tab = x2[:, :]
                else:
                    tab = h_full[:, :].rearrange("(a b) d -> a (b d)", b=2)
                wl_sb, wr_sb, br_sb = w_sb[L]
                side_sb = xT_sb if L == 0 else hT_sb
                for g in sorted(lm["groups"], key=lambda gg: -len(gg["windows"])):
                    NI = g["NI"]
                    nslab = NI // P
                    gt = gp.tile([P, max(nslab, 1), 2 * D], dt.bfloat16, tag="g0")
                    for s0 in range(0, nslab, SLAB_BUDGET):
                        sl = min(SLAB_BUDGET, nslab - s0)
                        nc.gpsimd.dma_gather(
                            out_ap=gt[:, s0:s0 + sl, :],
                            in_ap=tab,
                            idxs_ap=idx_sb[L][:, g["col"] + s0 * 8: g["col"] + (s0 + sl) * 8],
                            num_idxs=sl * P,
                            num_idxs_reg=sl * P,
                            elem_size=2 * D,
                            transpose=False,
                            single_packet=False,
                        )
                    for wi in g["windows"]:
                        b = wi["w"]
                        blk = slice(b * P, (b + 1) * P)
                        chunks = wi["chunks"]
                        ps = psa.tile([P, P], dt.float32)
                        for ci, (slab, t_g) in enumerate(chunks):
                            for par in (0, 1):
                                oh = ohp.tile([P, P], dt.bfloat16, tag="oh")
                                nc.vector.tensor_tensor(
                                    out=oh[:],
                                    in0=dstl_sb[L][:, 2 * t_g + par:2 * t_g + par + 1].to_broadcast([P, P]),
                                    in1=iota_sb[:],
                                    op=ALU.is_equal,
                                )
                                nc.tensor.matmul(
                                    out=ps[:], lhsT=oh[:],
                                    rhs=gt[:, slab, par * D:(par + 1) * D],
                                    start=(ci == 0 and par == 0),
                                    stop=(ci == len(chunks) - 1 and par == 1),
                                )
                        mean = wp.tile([P, P], dt.bfloat16, tag="mean")
                        nc.vector.tensor_tensor(
                            out=mean[:], in0=ps[:],
                            in1=invc_sb[:, b:b + 1].to_broadcast([P, P]),
                            op=ALU.mult,
                        )
                        tp = pst.tile([P, P], dt.bfloat16, tag="tp")
                        nc.tensor.transpose(out=tp[:], in_=mean[:], identity=ident[:])
                        meanT = wp.tile([P, P], dt.bfloat16, tag="meanT")
                        nc.vector.tensor_copy(meanT[:], tp[:])
                        po = pso.tile([P, P], dt.float32)
                        nc.tensor.matmul(out=po[:], lhsT=meanT[:], rhs=wl_sb[:], start=True, stop=False)
                        nc.tensor.matmul(out=po[:], lhsT=side_sb[:, blk], rhs=wr_sb[:], start=False, stop=False)
                        nc.tensor.matmul(out=po[:], lhsT=ones_sb[:], rhs=br_sb[:], start=False, stop=True)
                        if L == 0:
                            t1 = wp.tile([P, P], dt.bfloat16, tag="t1")
                            nc.scalar.activation(out=t1[:], in_=po[:], func=AF.Relu, bias=0.0, scale=1.0)
                            mk = wp.tile([P, P], dt.bfloat16, tag="mk")
                            nc.sync.dma_start(out=mk[:], in_=m2r[blk, :])
                            hr = wp.tile([P, P], dt.bfloat16, tag="hr")
                            nc.vector.tensor_tensor(out=hr[:], in0=t1[:], in1=mk[:], op=ALU.mult)
                            nc.sync.dma_start(out=cc_in[blk, :], in_=hr[:])
                            tp2 = pst.tile([P, P], dt.bfloat16, tag="tp")
                            nc.tensor.transpose(out=tp2[:], in_=hr[:], identity=ident[:])
                            nc.vector.tensor_copy(hT_sb[:, blk], tp2[:])
                        else:
                            o = wp.tile([P, P], dt.float32, tag="o")
                            nc.scalar.activation(out=o[:], in_=po[:], func=AF.Copy, bias=0.0, scale=1.0)
                            nc.sync.dma_start(out=out[blk, :], in_=o[:])
                if L == 0:
                    nc.gpsimd.collective_compute(
                        "AllGather",
                        mybir.AluOpType.bypass,
                        ins=[cc_in[:, :]],
                        outs=[h_full[:, :]],
                        replica_groups=[list(range(ncores))],
                    )
    nc.compile()
    return nc


# --------------------------------------------------------------------------
# Input map construction + host post-processing
# --------------------------------------------------------------------------

def make_in_maps(meta, x, mask, W1_l, b1_l, W1_r, W2_l, b2_l, W2_r):
    m = meta
    N, ncores, npad = m["N"], m["ncores"], m["npad"]
    s = m["split1"]
    xb = np.asarray(x, np.float32).astype(BF16)
    x2 = np.zeros((N // 2 + 1, 2 * D), BF16)
    x2[:N // 2] = xb.reshape(N // 2, 2 * D)
    x2 = np.ascontiguousarray(x2)
    w1lb = np.ascontiguousarray(np.asarray(W1_l, np.float32).astype(BF16))
    w1rb = np.ascontiguousarray(np.asarray(W1_r, np.float32).astype(BF16))
    w2lb = np.ascontiguousarray(np.asarray(W2_l, np.float32).astype(BF16))
    w2rb = np.ascontiguousarray(np.asarray(W2_r, np.float32).astype(BF16))
    b1c = np.ascontiguousarray(np.asarray(b1_l, np.float32).astype(BF16).reshape(1, P))
    b2c = np.ascontiguousarray(np.asarray(b2_l, np.float32).astype(BF16).reshape(1, P))
    iota = np.ascontiguousarray(
        np.broadcast_to(np.arange(P, dtype=np.float32), (P, P)).astype(BF16))
    onesv = np.ones((1, P), BF16)
    mask2 = np.asarray(mask, np.float32) * 2.0

    maps = []
    for c in range(ncores):
        ids = m["perm"][c]
        valid = ids >= 0
        safe = np.where(valid, ids, 0)
        xp = xb[safe]
        xp[~valid] = 0
        mp = mask2[safe].astype(BF16)
        mp[~valid] = 0
        inv = m["invcnt"][safe].copy()
        inv[~valid] = 1.0
        maps.append({
            "x2": x2,
            "idx1": m["idx"][0][c], "idx2": m["idx"][1][c],
            "dstl1": m["dstl"][0][c], "dstl2": m["dstl"][1][c],
            "xT": np.ascontiguousarray(xp.T),
            "m2r": np.ascontiguousarray(mp),
            "invc": np.ascontiguousarray(inv.reshape(m["blocks"], P).T),
            "iota": iota, "onesr": onesv,
            "w1l": w1lb, "w1r": w1rb, "w2l": w2lb, "w2r": w2rb,
            "b1r": b1c, "b2r": b2c,
        })
    return maps


def assemble_output(meta, results):
    m = meta
    out = np.empty((m["N"], D), np.float32)
    for c in range(m["ncores"]):
        o = np.asarray(results[c]["out"], np.float32)
        ids = m["perm"][c]
        valid = ids >= 0
        out[ids[valid]] = o[valid]
    return out


# --------------------------------------------------------------------------
# Entry point
# --------------------------------------------------------------------------

def _ensure_ntff_hook():
    """Reconstruct the axon NTFF profile hook if the image lacks
    antenv.axon_hooks (degraded boot). Needed only for trace=True."""
    import types
    try:
        from antenv.axon_hooks import get_axon_ntff_profile_hook
        if get_axon_ntff_profile_hook() is not None:
            return
    except ImportError:
        mod = types.ModuleType("antenv.axon_hooks")
        holder = [None]
        mod.set_axon_ntff_profile_hook = lambda h: holder.__setitem__(0, h)
        mod.get_axon_ntff_profile_hook = lambda: holder[0]
        sys.modules["antenv.axon_hooks"] = mod
        import antenv
        antenv.axon_hooks = mod
    if "/root/.axon_site" not in sys.path:
        sys.path.insert(0, "/root/.axon_site")
    from trn_agent_boot.trn_boot import _ntff_profile_via_ctypes
    from antenv.axon_hooks import set_axon_ntff_profile_hook
    hook = _ntff_profile_via_ctypes("/opt/axon/libaxon_pjrt.so")
    set_axon_ntff_profile_hook(hook)


_CACHE = {}


def _get_ctx(edge_index, N, ncores=8):
    ei = np.asarray(edge_index, np.int64)
    key = (N, ncores, hashlib.sha1(ei.tobytes()).hexdigest())
    ctx = _CACHE.get(key)
    if ctx is None:
        meta = build_meta(ei[0], ei[1], N, ncores)
        nc = build_nc(meta)
        _CACHE.clear()
        _CACHE[key] = ctx = (meta, nc)
    return ctx


def kernel(x, edge_index, drop_mask, W1_l, b1_l, W1_r, W2_l, b2_l, W2_r,
           trace=False):
    x = np.asarray(x, np.float32)
    meta, nc = _get_ctx(edge_index, x.shape[0])
    in_maps = make_in_maps(meta, x, drop_mask, W1_l, b1_l, W1_r, W2_l, b2_l, W2_r)
    if trace:
        _ensure_ntff_hook()
    from concourse.bass_utils import run_bass_kernel_spmd
    res = run_bass_kernel_spmd(
        nc, in_maps, core_ids=list(range(meta["ncores"])), trace=trace,
    )
    out = assemble_output(meta, res.results)
    if trace:
        return out, res
    return out
